# revision 42
# baseline (speedup 1.0000x reference)
"""Trainium2 Bass kernel for nn_CustomizedRelaModule (gnn_message_passing).

Math (after folding the deterministic permutation skeleton + adj into the
per-channel input weights):
    W[j, c, h] = adj[j, c] * w_in[c, k(j,c), h]   (0 at j == c)
    z[n, c, :] = data[n, :] @ W[:, c, :] + b_in[c]
    out[n, c]  = sum_h tanh(z)[n, c, h] * (neurons[h, c] * w_out[c, h, 0]) + b_out[c]
    returns (adj, out)

Sharding: channel-parallel — each of the 8 cores computes 32 of the 256
channels for all 4096 samples. data.T is replicated to every core (2MB in
fp16, streamed in 512-col pieces), per-channel weights are sliced per core.

Per-core device kernel: for each 1024-wide block of samples,
  z (128p x 1024) = two fp16 matmuls (K=256 split in 2) into fp32 PSUM
  h = tanh(z + b_in) on ScalarE (PSUM -> SBUF, fp16)
  out (32p x 1024) += G^T @ h   (gate+w_out folded into a (128,32) stationary)
then + b_out on VectorE and DMA out.
"""

import sys

if "/opt/trn_rl_repo" not in sys.path:
    sys.path.insert(0, "/opt/trn_rl_repo")

import numpy as np

N, V, NH = 4096, 256, 64
NCORES = 8
CPC = V // NCORES            # 32 channels per core
M_PER_CORE = CPC * NH        # 2048 (c_local, h) columns
MT = M_PER_CORE // 128       # 16 m-tiles (2 channels each)
NBLK = 1024                  # sample columns per block (2 PSUM banks)
NB = N // NBLK               # 4 blocks

_CACHE = {}

WARMUP_MMS = 10          # dummy matmuls to warm the PE HAM during input DMA


def _build_program():
    import concourse.bacc as bacc
    import concourse.tile as tile
    import concourse.mybir as mybir
    from concourse.tile import add_dep_helper

    f32 = mybir.dt.float32
    f16 = mybir.dt.float16
    Tanh = mybir.ActivationFunctionType.Tanh
    Ident = mybir.ActivationFunctionType.Identity

    nc = bacc.Bacc("TRN2", target_bir_lowering=False, debug=False,
                   num_devices=NCORES)

    x2_d = nc.dram_tensor("x2", [128, 2 * N], f16, kind="ExternalInput").ap()
    w_d = nc.dram_tensor("w", [V, M_PER_CORE], f16, kind="ExternalInput").ap()
    g_d = nc.dram_tensor("g", [128, MT * CPC], f16, kind="ExternalInput").ap()
    b_d = nc.dram_tensor("b", [128, MT], f32, kind="ExternalInput").ap()
    bo_d = nc.dram_tensor("bo", [CPC, 1], f32, kind="ExternalInput").ap()
    out_d = nc.dram_tensor("out", [CPC, N], f32, kind="ExternalOutput").ap()

    with tile.TileContext(nc) as tc:
        with tc.tile_pool(name="const", bufs=1) as constp, \
             tc.tile_pool(name="xin", bufs=4) as xp, \
             tc.tile_pool(name="warm", bufs=1) as warmp, \
             tc.tile_pool(name="zpsum", bufs=3, space="PSUM") as zp, \
             tc.tile_pool(name="redpsum", bufs=1, space="PSUM") as rp, \
             tc.tile_pool(name="hbuf", bufs=9) as hp, \
             tc.tile_pool(name="obuf", bufs=2) as op:

            # ── PE warm-up + early tanh table load. Dummy matmuls fill the
            # initial DMA wait and flip HAM to 2.4 GHz; the tiny activation
            # forces ACT_TABLE_LOAD to run now instead of at first real tanh.
            wt_w = warmp.tile([128, 512], f16, tag="warmdat")
            nc.vector.memset(wt_w[:], 0.0)
            if WARMUP_MMS:
                wt_ps = zp.tile([128, NBLK], f32, tag="z", name="warm_ps")
                for _ in range(WARMUP_MMS):
                    nc.tensor.matmul(wt_ps[:, 0:512], lhsT=wt_w[:, 0:128],
                                     rhs=wt_w[:], start=True, stop=True)

            # ── startup DMAs. x is host-packed so one 256KB DMA delivers
            # both K-halves of a 512-sample piece. First z group needs
            # x(0,0) + w0c0 + w1c0: spread across the gpsimd/sync/scalar
            # rings (each ring's transfers serialize at ~110GB/s).
            def xtile(blk, half):
                t = xp.tile([128, NBLK], f16, tag=f"xh{half}",
                            name=f"x_{blk}_{half}")
                return t

            def xsrc(blk, half):
                cb = blk * 2 + half
                return x2_d[:, cb * NBLK:(cb + 1) * NBLK]

            # block-0 x pieces split by K-half and interleaved with the w
            # chunks across both HWDGE rings, in exact first-use order
            # (subtile deps let the first matmul start on the kt0 piece)
            x_tiles = []
            t0 = xtile(0, 0)
            t1 = xtile(0, 1)
            x_tiles.append((0, t0))
            x_tiles.append((1, t1))
            w_sb = [constp.tile([128, M_PER_CORE], f16, tag=f"w{kt}",
                                name=f"w_sb{kt}")
                    for kt in range(2)]
            WCH = 512
            nc.sync.dma_start(t0[:, 0:512], x2_d[:, 0:512])
            nc.scalar.dma_start(t0[:, 512:NBLK], x2_d[:, 512:NBLK])
            nc.sync.dma_start(w_sb[0][:, 0:WCH], w_d[0:128, 0:WCH])
            nc.scalar.dma_start(w_sb[1][:, 0:WCH], w_d[128:256, 0:WCH])
            nc.sync.dma_start(t1[:, 0:512], x2_d[:, NBLK:NBLK + 512])
            nc.scalar.dma_start(t1[:, 512:NBLK], x2_d[:, NBLK + 512:2 * NBLK])
            for ch in range(1, M_PER_CORE // WCH):
                csl = slice(ch * WCH, (ch + 1) * WCH)
                nc.sync.dma_start(w_sb[0][:, csl], w_d[0:128, csl])
                nc.scalar.dma_start(w_sb[1][:, csl], w_d[128:256, csl])
            b_sb = constp.tile([128, MT], f32, tag="b")
            nc.gpsimd.dma_start(b_sb[:], b_d[:])
            bo_sb = constp.tile([CPC, 1], f32, tag="bo")
            nc.gpsimd.dma_start(bo_sb[:], bo_d[:])
            g_sb = constp.tile([128, MT * CPC], f16, tag="g")
            nc.gpsimd.dma_start(g_sb[:], g_d[:])

            # reduction matmuls run RED_LAG mt-steps behind their tanh so
            # the PE never waits on ScalarE
            RED_LAG = 3
            pending_red = []

            def emit_red(pr):
                h, gsl, first, last, red_t, pblk, pnsl_d = pr
                for half in range(NBLK // 512):
                    nsl = slice(half * 512, (half + 1) * 512)
                    nc.tensor.matmul(red_t[:, nsl],
                                     lhsT=g_sb[:, gsl],
                                     rhs=h[:, nsl],
                                     start=first, stop=last)
                if last:
                    o = op.tile([CPC, NBLK], f32, tag="o")
                    nc.vector.tensor_scalar_add(o[:], red_t[:], bo_sb[:, 0:1])
                    nc.sync.dma_start(out_d[:, pnsl_d], o[:])

            for blk in range(NB):
                nsl_d = slice(blk * NBLK, (blk + 1) * NBLK)
                xr = x_tiles
                red = rp.tile([CPC, NBLK], f32)
                for mt in range(MT):
                    msl = slice(mt * 128, (mt + 1) * 128)
                    z = zp.tile([128, NBLK], f32, tag="z")
                    xmap = {hf: t for hf, t in xr}
                    for half in range(NBLK // 512):
                        nsl = slice(half * 512, (half + 1) * 512)
                        nc.tensor.matmul(z[:, nsl],
                                         lhsT=w_sb[0][:, msl],
                                         rhs=xmap[half][:, 0:512],
                                         start=True, stop=False)
                        nc.tensor.matmul(z[:, nsl],
                                         lhsT=w_sb[1][:, msl],
                                         rhs=xmap[half][:, 512:NBLK],
                                         start=False, stop=True)
                    # drain reds in batches of 4 so the PE pays the w<->g
                    # stationary-switch cost only twice per 4 mt-steps
                    batch = 4
                    if len(pending_red) >= RED_LAG + batch - 1:
                        for _ in range(batch):
                            emit_red(pending_red.pop(0))
                    h = hp.tile([128, NBLK], f16)
                    act_inst = nc.scalar.activation(h[:], z[:], Tanh,
                                                    bias=b_sb[:, mt:mt + 1])
                    if mt == 1:
                        prefetch_gate = act_inst
                    pending_red.append((h, slice(mt * CPC, (mt + 1) * CPC),
                                        mt == 0, mt == MT - 1, red, blk,
                                        nsl_d))

                # prefetch next block's x on the gpsimd queue
                if blk + 1 < NB:
                    x_tiles = []
                    for half in range(2):
                        t = xtile(blk + 1, half)
                        dma = nc.gpsimd.dma_start(t[:], xsrc(blk + 1, half))
                        # hold the prefetch back so it can't steal HBM
                        # bandwidth from startup-critical loads
                        add_dep_helper(dma.ins, prefetch_gate.ins,
                                       reason="x prefetch after block start")
                        x_tiles.append((half, t))

            for pr in pending_red:
                emit_red(pr)
            pending_red = []

    nc.compile()
    return nc


def _get_program():
    if "nc" not in _CACHE:
        _CACHE["nc"] = _build_program()
    return _CACHE["nc"]


def _make_in_maps(data, adj, w_in, b_in, w_out, neurons, b_out):
    xT = data.T.astype(np.float16)  # (V, N)
    # pack per-(block,half) chunks as [kt0 512 cols | kt1 512 cols]
    x2 = np.empty((128, 2 * N), dtype=np.float16)
    for blk in range(NB):
        for half in range(2):
            cb = blk * 2 + half
            seg = slice(blk * NBLK + half * 512, blk * NBLK + (half + 1) * 512)
            x2[:, cb * NBLK:cb * NBLK + 512] = xT[0:128, seg]
            x2[:, cb * NBLK + 512:(cb + 1) * NBLK] = xT[128:256, seg]
    x2 = np.ascontiguousarray(x2)

    # Fold adj + permutation skeleton into dense per-channel weights.
    W = np.zeros((V, V, NH), dtype=np.float32)  # (j, c, h)
    for c in range(V):
        ac = adj[:, c]
        W[:c, c, :] = w_in[c, :c, :] * ac[:c, None]
        W[c + 1:, c, :] = w_in[c, c:, :] * ac[c + 1:, None]

    gate = neurons.T * w_out[:, :, 0]  # (V, NH)

    in_maps = []
    for k in range(NCORES):
        cs = slice(CPC * k, CPC * (k + 1))
        Wk = np.ascontiguousarray(
            W[:, cs, :].reshape(V, M_PER_CORE).astype(np.float16))
        gk = gate[cs]     # (32, 64)
        bik = b_in[cs]    # (32, 64)
        Gk = np.zeros((128, MT * CPC), dtype=np.float32)
        bk = np.zeros((128, MT), dtype=np.float32)
        for mt in range(MT):
            for i in range(2):
                cl = 2 * mt + i
                Gk[i * NH:(i + 1) * NH, mt * CPC + cl] = gk[cl]
                bk[i * NH:(i + 1) * NH, mt] = bik[cl]
        bok = np.ascontiguousarray(b_out[cs].reshape(CPC, 1).astype(np.float32))
        in_maps.append({"x2": x2, "w": Wk, "g": Gk.astype(np.float16),
                        "b": bk, "bo": bok})
    return in_maps


def _run(in_maps, trace=False, tmpdir=None):
    from concourse.bass_utils import run_bass_kernel_spmd
    nc = _get_program()
    return run_bass_kernel_spmd(nc, in_maps, core_ids=list(range(NCORES)),
                                trace=trace, tmpdir=tmpdir)


def kernel(data, adj, neurons, w_in, b_in, w_out, b_out, perm):
    data = np.asarray(data, dtype=np.float32)
    adj = np.asarray(adj, dtype=np.float32)
    neurons = np.asarray(neurons, dtype=np.float32)
    w_in = np.asarray(w_in, dtype=np.float32)
    b_in = np.asarray(b_in, dtype=np.float32)
    w_out = np.asarray(w_out, dtype=np.float32)
    b_out = np.asarray(b_out, dtype=np.float32)
    assert data.shape == (N, V)

    in_maps = _make_in_maps(data, adj, w_in, b_in, w_out, neurons, b_out)
    res = _run(in_maps)

    out = np.empty((N, V), dtype=np.float32)
    for k in range(NCORES):
        out[:, CPC * k:CPC * (k + 1)] = res.results[k]["out"].T
    return adj, out


# revision 43
# speedup vs baseline: 1.0050x; 1.0050x over previous
"""Trainium2 Bass kernel for nn_CustomizedRelaModule (gnn_message_passing).

Math (after folding the deterministic permutation skeleton + adj into the
per-channel input weights):
    W[j, c, h] = adj[j, c] * w_in[c, k(j,c), h]   (0 at j == c)
    z[n, c, :] = data[n, :] @ W[:, c, :] + b_in[c]
    out[n, c]  = sum_h tanh(z)[n, c, h] * (neurons[h, c] * w_out[c, h, 0]) + b_out[c]
    returns (adj, out)

Sharding: channel-parallel — each of the 8 cores computes 32 of the 256
channels for all 4096 samples. data.T is replicated to every core (2MB in
fp16, streamed in 512-col pieces), per-channel weights are sliced per core.

Per-core device kernel: for each 1024-wide block of samples,
  z (128p x 1024) = two fp16 matmuls (K=256 split in 2) into fp32 PSUM
  h = tanh(z + b_in) on ScalarE (PSUM -> SBUF, fp16)
  out (32p x 1024) += G^T @ h   (gate+w_out folded into a (128,32) stationary)
then + b_out on VectorE and DMA out.
"""

import sys

if "/opt/trn_rl_repo" not in sys.path:
    sys.path.insert(0, "/opt/trn_rl_repo")

import numpy as np

N, V, NH = 4096, 256, 64
NCORES = 8
CPC = V // NCORES            # 32 channels per core
M_PER_CORE = CPC * NH        # 2048 (c_local, h) columns
MT = M_PER_CORE // 128       # 16 m-tiles (2 channels each)
NBLK = 1024                  # sample columns per block (2 PSUM banks)
NB = N // NBLK               # 4 blocks

_CACHE = {}

WARMUP_MMS = 16          # dummy matmuls to warm the PE HAM during input DMA


def _build_program():
    import concourse.bacc as bacc
    import concourse.tile as tile
    import concourse.mybir as mybir
    from concourse.tile import add_dep_helper

    f32 = mybir.dt.float32
    f16 = mybir.dt.float16
    Tanh = mybir.ActivationFunctionType.Tanh
    Ident = mybir.ActivationFunctionType.Identity

    nc = bacc.Bacc("TRN2", target_bir_lowering=False, debug=False,
                   num_devices=NCORES)

    x2_d = nc.dram_tensor("x2", [128, 2 * N], f16, kind="ExternalInput").ap()
    w_d = nc.dram_tensor("w", [V, M_PER_CORE], f16, kind="ExternalInput").ap()
    g_d = nc.dram_tensor("g", [128, MT * CPC], f16, kind="ExternalInput").ap()
    b_d = nc.dram_tensor("b", [128, MT], f32, kind="ExternalInput").ap()
    bo_d = nc.dram_tensor("bo", [CPC, 1], f32, kind="ExternalInput").ap()
    out_d = nc.dram_tensor("out", [CPC, N], f32, kind="ExternalOutput").ap()

    with tile.TileContext(nc) as tc:
        with tc.tile_pool(name="const", bufs=1) as constp, \
             tc.tile_pool(name="xin", bufs=4) as xp, \
             tc.tile_pool(name="warm", bufs=1) as warmp, \
             tc.tile_pool(name="zpsum", bufs=3, space="PSUM") as zp, \
             tc.tile_pool(name="redpsum", bufs=1, space="PSUM") as rp, \
             tc.tile_pool(name="hbuf", bufs=9) as hp, \
             tc.tile_pool(name="obuf", bufs=2) as op:

            # ── PE warm-up + early tanh table load. Dummy matmuls fill the
            # initial DMA wait and flip HAM to 2.4 GHz; the tiny activation
            # forces ACT_TABLE_LOAD to run now instead of at first real tanh.
            wt_w = warmp.tile([128, 512], f16, tag="warmdat")
            nc.vector.memset(wt_w[:], 0.0)
            if WARMUP_MMS:
                wt_ps = zp.tile([128, NBLK], f32, tag="z", name="warm_ps")
                for _ in range(WARMUP_MMS):
                    nc.tensor.matmul(wt_ps[:, 0:512], lhsT=wt_w[:, 0:128],
                                     rhs=wt_w[:], start=True, stop=True)

            # ── startup DMAs. x is host-packed so one 256KB DMA delivers
            # both K-halves of a 512-sample piece. First z group needs
            # x(0,0) + w0c0 + w1c0: spread across the gpsimd/sync/scalar
            # rings (each ring's transfers serialize at ~110GB/s).
            def xtile(blk, half):
                t = xp.tile([128, NBLK], f16, tag=f"xh{half}",
                            name=f"x_{blk}_{half}")
                return t

            def xsrc(blk, half):
                cb = blk * 2 + half
                return x2_d[:, cb * NBLK:(cb + 1) * NBLK]

            # block-0 x pieces split by K-half and interleaved with the w
            # chunks across both HWDGE rings, in exact first-use order
            # (subtile deps let the first matmul start on the kt0 piece)
            x_tiles = []
            t0 = xtile(0, 0)
            t1 = xtile(0, 1)
            x_tiles.append((0, t0))
            x_tiles.append((1, t1))
            w_sb = [constp.tile([128, M_PER_CORE], f16, tag=f"w{kt}",
                                name=f"w_sb{kt}")
                    for kt in range(2)]
            WCH = 1024
            nc.sync.dma_start(t0[:, 0:512], x2_d[:, 0:512])
            nc.scalar.dma_start(t0[:, 512:NBLK], x2_d[:, 512:NBLK])
            nc.sync.dma_start(w_sb[0][:, 0:WCH], w_d[0:128, 0:WCH])
            nc.scalar.dma_start(w_sb[1][:, 0:WCH], w_d[128:256, 0:WCH])
            nc.sync.dma_start(t1[:, 0:512], x2_d[:, NBLK:NBLK + 512])
            nc.scalar.dma_start(t1[:, 512:NBLK], x2_d[:, NBLK + 512:2 * NBLK])
            for ch in range(1, M_PER_CORE // WCH):
                csl = slice(ch * WCH, (ch + 1) * WCH)
                nc.sync.dma_start(w_sb[0][:, csl], w_d[0:128, csl])
                nc.scalar.dma_start(w_sb[1][:, csl], w_d[128:256, csl])
            b_sb = constp.tile([128, MT], f32, tag="b")
            nc.gpsimd.dma_start(b_sb[:], b_d[:])
            bo_sb = constp.tile([CPC, 1], f32, tag="bo")
            nc.gpsimd.dma_start(bo_sb[:], bo_d[:])
            g_sb = constp.tile([128, MT * CPC], f16, tag="g")
            nc.gpsimd.dma_start(g_sb[:], g_d[:])

            # reduction matmuls run RED_LAG mt-steps behind their tanh so
            # the PE never waits on ScalarE
            RED_LAG = 3
            pending_red = []

            def emit_red(pr):
                h, gsl, first, last, red_t, pblk, pnsl_d = pr
                for half in range(NBLK // 512):
                    nsl = slice(half * 512, (half + 1) * 512)
                    nc.tensor.matmul(red_t[:, nsl],
                                     lhsT=g_sb[:, gsl],
                                     rhs=h[:, nsl],
                                     start=first, stop=last)
                if last:
                    o = op.tile([CPC, NBLK], f32, tag="o")
                    nc.vector.tensor_scalar_add(o[:], red_t[:], bo_sb[:, 0:1])
                    nc.sync.dma_start(out_d[:, pnsl_d], o[:])

            for blk in range(NB):
                nsl_d = slice(blk * NBLK, (blk + 1) * NBLK)
                xr = x_tiles
                red = rp.tile([CPC, NBLK], f32)
                for mt in range(MT):
                    msl = slice(mt * 128, (mt + 1) * 128)
                    z = zp.tile([128, NBLK], f32, tag="z")
                    xmap = {hf: t for hf, t in xr}
                    for half in range(NBLK // 512):
                        nsl = slice(half * 512, (half + 1) * 512)
                        nc.tensor.matmul(z[:, nsl],
                                         lhsT=w_sb[0][:, msl],
                                         rhs=xmap[half][:, 0:512],
                                         start=True, stop=False)
                        nc.tensor.matmul(z[:, nsl],
                                         lhsT=w_sb[1][:, msl],
                                         rhs=xmap[half][:, 512:NBLK],
                                         start=False, stop=True)
                    # drain reds in batches of 4 so the PE pays the w<->g
                    # stationary-switch cost only twice per 4 mt-steps
                    batch = 4
                    if len(pending_red) >= RED_LAG + batch - 1:
                        for _ in range(batch):
                            emit_red(pending_red.pop(0))
                    h = hp.tile([128, NBLK], f16)
                    act_inst = nc.scalar.activation(h[:], z[:], Tanh,
                                                    bias=b_sb[:, mt:mt + 1])
                    if mt == 1:
                        prefetch_gate = act_inst
                    pending_red.append((h, slice(mt * CPC, (mt + 1) * CPC),
                                        mt == 0, mt == MT - 1, red, blk,
                                        nsl_d))

                # prefetch next block's x on the gpsimd queue
                if blk + 1 < NB:
                    x_tiles = []
                    for half in range(2):
                        t = xtile(blk + 1, half)
                        dma = nc.gpsimd.dma_start(t[:], xsrc(blk + 1, half))
                        # hold the prefetch back so it can't steal HBM
                        # bandwidth from startup-critical loads
                        add_dep_helper(dma.ins, prefetch_gate.ins,
                                       reason="x prefetch after block start")
                        x_tiles.append((half, t))

            for pr in pending_red:
                emit_red(pr)
            pending_red = []

    nc.compile()
    return nc


def _get_program():
    if "nc" not in _CACHE:
        _CACHE["nc"] = _build_program()
    return _CACHE["nc"]


def _make_in_maps(data, adj, w_in, b_in, w_out, neurons, b_out):
    xT = data.T.astype(np.float16)  # (V, N)
    # pack per-(block,half) chunks as [kt0 512 cols | kt1 512 cols]
    x2 = np.empty((128, 2 * N), dtype=np.float16)
    for blk in range(NB):
        for half in range(2):
            cb = blk * 2 + half
            seg = slice(blk * NBLK + half * 512, blk * NBLK + (half + 1) * 512)
            x2[:, cb * NBLK:cb * NBLK + 512] = xT[0:128, seg]
            x2[:, cb * NBLK + 512:(cb + 1) * NBLK] = xT[128:256, seg]
    x2 = np.ascontiguousarray(x2)

    # Fold adj + permutation skeleton into dense per-channel weights.
    W = np.zeros((V, V, NH), dtype=np.float32)  # (j, c, h)
    for c in range(V):
        ac = adj[:, c]
        W[:c, c, :] = w_in[c, :c, :] * ac[:c, None]
        W[c + 1:, c, :] = w_in[c, c:, :] * ac[c + 1:, None]

    gate = neurons.T * w_out[:, :, 0]  # (V, NH)

    in_maps = []
    for k in range(NCORES):
        cs = slice(CPC * k, CPC * (k + 1))
        Wk = np.ascontiguousarray(
            W[:, cs, :].reshape(V, M_PER_CORE).astype(np.float16))
        gk = gate[cs]     # (32, 64)
        bik = b_in[cs]    # (32, 64)
        Gk = np.zeros((128, MT * CPC), dtype=np.float32)
        bk = np.zeros((128, MT), dtype=np.float32)
        for mt in range(MT):
            for i in range(2):
                cl = 2 * mt + i
                Gk[i * NH:(i + 1) * NH, mt * CPC + cl] = gk[cl]
                bk[i * NH:(i + 1) * NH, mt] = bik[cl]
        bok = np.ascontiguousarray(b_out[cs].reshape(CPC, 1).astype(np.float32))
        in_maps.append({"x2": x2, "w": Wk, "g": Gk.astype(np.float16),
                        "b": bk, "bo": bok})
    return in_maps


def _run(in_maps, trace=False, tmpdir=None):
    from concourse.bass_utils import run_bass_kernel_spmd
    nc = _get_program()
    return run_bass_kernel_spmd(nc, in_maps, core_ids=list(range(NCORES)),
                                trace=trace, tmpdir=tmpdir)


def kernel(data, adj, neurons, w_in, b_in, w_out, b_out, perm):
    data = np.asarray(data, dtype=np.float32)
    adj = np.asarray(adj, dtype=np.float32)
    neurons = np.asarray(neurons, dtype=np.float32)
    w_in = np.asarray(w_in, dtype=np.float32)
    b_in = np.asarray(b_in, dtype=np.float32)
    w_out = np.asarray(w_out, dtype=np.float32)
    b_out = np.asarray(b_out, dtype=np.float32)
    assert data.shape == (N, V)

    in_maps = _make_in_maps(data, adj, w_in, b_in, w_out, neurons, b_out)
    res = _run(in_maps)

    out = np.empty((N, V), dtype=np.float32)
    for k in range(NCORES):
        out[:, CPC * k:CPC * (k + 1)] = res.results[k]["out"].T
    return adj, out


# revision 44
# speedup vs baseline: 1.0079x; 1.0029x over previous
"""Trainium2 Bass kernel for nn_CustomizedRelaModule (gnn_message_passing).

Math (after folding the deterministic permutation skeleton + adj into the
per-channel input weights):
    W[j, c, h] = adj[j, c] * w_in[c, k(j,c), h]   (0 at j == c)
    z[n, c, :] = data[n, :] @ W[:, c, :] + b_in[c]
    out[n, c]  = sum_h tanh(z)[n, c, h] * (neurons[h, c] * w_out[c, h, 0]) + b_out[c]
    returns (adj, out)

Sharding: channel-parallel — each of the 8 cores computes 32 of the 256
channels for all 4096 samples. data.T is replicated to every core (2MB in
fp16, streamed in 512-col pieces), per-channel weights are sliced per core.

Per-core device kernel: for each 1024-wide block of samples,
  z (128p x 1024) = two fp16 matmuls (K=256 split in 2) into fp32 PSUM
  h = tanh(z + b_in) on ScalarE (PSUM -> SBUF, fp16)
  out (32p x 1024) += G^T @ h   (gate+w_out folded into a (128,32) stationary)
then + b_out on VectorE and DMA out.
"""

import sys

if "/opt/trn_rl_repo" not in sys.path:
    sys.path.insert(0, "/opt/trn_rl_repo")

import numpy as np

N, V, NH = 4096, 256, 64
NCORES = 8
CPC = V // NCORES            # 32 channels per core
M_PER_CORE = CPC * NH        # 2048 (c_local, h) columns
MT = M_PER_CORE // 128       # 16 m-tiles (2 channels each)
NBLK = 1024                  # sample columns per block (2 PSUM banks)
NB = N // NBLK               # 4 blocks

_CACHE = {}

WARMUP_MMS = 14          # dummy matmuls to warm the PE HAM during input DMA


def _build_program():
    import concourse.bacc as bacc
    import concourse.tile as tile
    import concourse.mybir as mybir
    from concourse.tile import add_dep_helper

    f32 = mybir.dt.float32
    f16 = mybir.dt.float16
    Tanh = mybir.ActivationFunctionType.Tanh
    Ident = mybir.ActivationFunctionType.Identity

    nc = bacc.Bacc("TRN2", target_bir_lowering=False, debug=False,
                   num_devices=NCORES)

    x2_d = nc.dram_tensor("x2", [128, 2 * N], f16, kind="ExternalInput").ap()
    w_d = nc.dram_tensor("w", [V, M_PER_CORE], f16, kind="ExternalInput").ap()
    g_d = nc.dram_tensor("g", [128, MT * CPC], f16, kind="ExternalInput").ap()
    b_d = nc.dram_tensor("b", [128, MT], f32, kind="ExternalInput").ap()
    bo_d = nc.dram_tensor("bo", [CPC, 1], f32, kind="ExternalInput").ap()
    out_d = nc.dram_tensor("out", [CPC, N], f32, kind="ExternalOutput").ap()

    with tile.TileContext(nc) as tc:
        with tc.tile_pool(name="const", bufs=1) as constp, \
             tc.tile_pool(name="xin", bufs=4) as xp, \
             tc.tile_pool(name="warm", bufs=1) as warmp, \
             tc.tile_pool(name="zpsum", bufs=3, space="PSUM") as zp, \
             tc.tile_pool(name="redpsum", bufs=1, space="PSUM") as rp, \
             tc.tile_pool(name="hbuf", bufs=9) as hp, \
             tc.tile_pool(name="obuf", bufs=2) as op:

            # ── PE warm-up + early tanh table load. Dummy matmuls fill the
            # initial DMA wait and flip HAM to 2.4 GHz; the tiny activation
            # forces ACT_TABLE_LOAD to run now instead of at first real tanh.
            wt_w = warmp.tile([128, 512], f16, tag="warmdat")
            nc.vector.memset(wt_w[:], 0.0)
            if WARMUP_MMS:
                wt_ps = zp.tile([128, NBLK], f32, tag="z", name="warm_ps")
                for _ in range(WARMUP_MMS):
                    nc.tensor.matmul(wt_ps[:, 0:512], lhsT=wt_w[:, 0:128],
                                     rhs=wt_w[:], start=True, stop=True)

            # ── startup DMAs. x is host-packed so one 256KB DMA delivers
            # both K-halves of a 512-sample piece. First z group needs
            # x(0,0) + w0c0 + w1c0: spread across the gpsimd/sync/scalar
            # rings (each ring's transfers serialize at ~110GB/s).
            def xtile(blk, half):
                t = xp.tile([128, NBLK], f16, tag=f"xh{half}",
                            name=f"x_{blk}_{half}")
                return t

            def xsrc(blk, half):
                cb = blk * 2 + half
                return x2_d[:, cb * NBLK:(cb + 1) * NBLK]

            # block-0 x pieces split by K-half and interleaved with the w
            # chunks across both HWDGE rings, in exact first-use order
            # (subtile deps let the first matmul start on the kt0 piece)
            x_tiles = []
            t0 = xtile(0, 0)
            t1 = xtile(0, 1)
            x_tiles.append((0, t0))
            x_tiles.append((1, t1))
            w_sb = [constp.tile([128, M_PER_CORE], f16, tag=f"w{kt}",
                                name=f"w_sb{kt}")
                    for kt in range(2)]
            WCH = 1024
            nc.sync.dma_start(t0[:, 0:512], x2_d[:, 0:512])
            nc.scalar.dma_start(t0[:, 512:NBLK], x2_d[:, 512:NBLK])
            nc.sync.dma_start(w_sb[0][:, 0:WCH], w_d[0:128, 0:WCH])
            nc.scalar.dma_start(w_sb[1][:, 0:WCH], w_d[128:256, 0:WCH])
            nc.sync.dma_start(t1[:, 0:512], x2_d[:, NBLK:NBLK + 512])
            nc.scalar.dma_start(t1[:, 512:NBLK], x2_d[:, NBLK + 512:2 * NBLK])
            for ch in range(1, M_PER_CORE // WCH):
                csl = slice(ch * WCH, (ch + 1) * WCH)
                nc.sync.dma_start(w_sb[0][:, csl], w_d[0:128, csl])
                nc.scalar.dma_start(w_sb[1][:, csl], w_d[128:256, csl])
            b_sb = constp.tile([128, MT], f32, tag="b")
            nc.gpsimd.dma_start(b_sb[:], b_d[:])
            bo_sb = constp.tile([CPC, 1], f32, tag="bo")
            nc.gpsimd.dma_start(bo_sb[:], bo_d[:])
            g_sb = constp.tile([128, MT * CPC], f16, tag="g")
            nc.gpsimd.dma_start(g_sb[:], g_d[:])

            # reduction matmuls run RED_LAG mt-steps behind their tanh so
            # the PE never waits on ScalarE
            RED_LAG = 3
            pending_red = []

            def emit_red(pr):
                h, gsl, first, last, red_t, pblk, pnsl_d = pr
                for half in range(NBLK // 512):
                    nsl = slice(half * 512, (half + 1) * 512)
                    nc.tensor.matmul(red_t[:, nsl],
                                     lhsT=g_sb[:, gsl],
                                     rhs=h[:, nsl],
                                     start=first, stop=last)
                if last:
                    o = op.tile([CPC, NBLK], f32, tag="o")
                    nc.vector.tensor_scalar_add(o[:], red_t[:], bo_sb[:, 0:1])
                    nc.sync.dma_start(out_d[:, pnsl_d], o[:])

            for blk in range(NB):
                nsl_d = slice(blk * NBLK, (blk + 1) * NBLK)
                xr = x_tiles
                red = rp.tile([CPC, NBLK], f32)
                for mt in range(MT):
                    msl = slice(mt * 128, (mt + 1) * 128)
                    z = zp.tile([128, NBLK], f32, tag="z")
                    xmap = {hf: t for hf, t in xr}
                    for half in range(NBLK // 512):
                        nsl = slice(half * 512, (half + 1) * 512)
                        nc.tensor.matmul(z[:, nsl],
                                         lhsT=w_sb[0][:, msl],
                                         rhs=xmap[half][:, 0:512],
                                         start=True, stop=False)
                        nc.tensor.matmul(z[:, nsl],
                                         lhsT=w_sb[1][:, msl],
                                         rhs=xmap[half][:, 512:NBLK],
                                         start=False, stop=True)
                    # drain reds in batches of 4 so the PE pays the w<->g
                    # stationary-switch cost only twice per 4 mt-steps
                    batch = 4
                    if len(pending_red) >= RED_LAG + batch - 1:
                        for _ in range(batch):
                            emit_red(pending_red.pop(0))
                    h = hp.tile([128, NBLK], f16)
                    act_inst = nc.scalar.activation(h[:], z[:], Tanh,
                                                    bias=b_sb[:, mt:mt + 1])
                    if mt == 1:
                        prefetch_gate = act_inst
                    pending_red.append((h, slice(mt * CPC, (mt + 1) * CPC),
                                        mt == 0, mt == MT - 1, red, blk,
                                        nsl_d))

                # prefetch next block's x on the gpsimd queue
                if blk + 1 < NB:
                    x_tiles = []
                    for half in range(2):
                        t = xtile(blk + 1, half)
                        dma = nc.gpsimd.dma_start(t[:], xsrc(blk + 1, half))
                        # hold the prefetch back so it can't steal HBM
                        # bandwidth from startup-critical loads
                        add_dep_helper(dma.ins, prefetch_gate.ins,
                                       reason="x prefetch after block start")
                        x_tiles.append((half, t))

            for pr in pending_red:
                emit_red(pr)
            pending_red = []

    nc.compile()
    return nc


def _get_program():
    if "nc" not in _CACHE:
        _CACHE["nc"] = _build_program()
    return _CACHE["nc"]


def _make_in_maps(data, adj, w_in, b_in, w_out, neurons, b_out):
    xT = data.T.astype(np.float16)  # (V, N)
    # pack per-(block,half) chunks as [kt0 512 cols | kt1 512 cols]
    x2 = np.empty((128, 2 * N), dtype=np.float16)
    for blk in range(NB):
        for half in range(2):
            cb = blk * 2 + half
            seg = slice(blk * NBLK + half * 512, blk * NBLK + (half + 1) * 512)
            x2[:, cb * NBLK:cb * NBLK + 512] = xT[0:128, seg]
            x2[:, cb * NBLK + 512:(cb + 1) * NBLK] = xT[128:256, seg]
    x2 = np.ascontiguousarray(x2)

    # Fold adj + permutation skeleton into dense per-channel weights.
    W = np.zeros((V, V, NH), dtype=np.float32)  # (j, c, h)
    for c in range(V):
        ac = adj[:, c]
        W[:c, c, :] = w_in[c, :c, :] * ac[:c, None]
        W[c + 1:, c, :] = w_in[c, c:, :] * ac[c + 1:, None]

    gate = neurons.T * w_out[:, :, 0]  # (V, NH)

    in_maps = []
    for k in range(NCORES):
        cs = slice(CPC * k, CPC * (k + 1))
        Wk = np.ascontiguousarray(
            W[:, cs, :].reshape(V, M_PER_CORE).astype(np.float16))
        gk = gate[cs]     # (32, 64)
        bik = b_in[cs]    # (32, 64)
        Gk = np.zeros((128, MT * CPC), dtype=np.float32)
        bk = np.zeros((128, MT), dtype=np.float32)
        for mt in range(MT):
            for i in range(2):
                cl = 2 * mt + i
                Gk[i * NH:(i + 1) * NH, mt * CPC + cl] = gk[cl]
                bk[i * NH:(i + 1) * NH, mt] = bik[cl]
        bok = np.ascontiguousarray(b_out[cs].reshape(CPC, 1).astype(np.float32))
        in_maps.append({"x2": x2, "w": Wk, "g": Gk.astype(np.float16),
                        "b": bk, "bo": bok})
    return in_maps


def _run(in_maps, trace=False, tmpdir=None):
    from concourse.bass_utils import run_bass_kernel_spmd
    nc = _get_program()
    return run_bass_kernel_spmd(nc, in_maps, core_ids=list(range(NCORES)),
                                trace=trace, tmpdir=tmpdir)


def kernel(data, adj, neurons, w_in, b_in, w_out, b_out, perm):
    data = np.asarray(data, dtype=np.float32)
    adj = np.asarray(adj, dtype=np.float32)
    neurons = np.asarray(neurons, dtype=np.float32)
    w_in = np.asarray(w_in, dtype=np.float32)
    b_in = np.asarray(b_in, dtype=np.float32)
    w_out = np.asarray(w_out, dtype=np.float32)
    b_out = np.asarray(b_out, dtype=np.float32)
    assert data.shape == (N, V)

    in_maps = _make_in_maps(data, adj, w_in, b_in, w_out, neurons, b_out)
    res = _run(in_maps)

    out = np.empty((N, V), dtype=np.float32)
    for k in range(NCORES):
        out[:, CPC * k:CPC * (k + 1)] = res.results[k]["out"].T
    return adj, out


# revision 45
# speedup vs baseline: 1.0083x; 1.0003x over previous
"""Trainium2 Bass kernel for nn_CustomizedRelaModule (gnn_message_passing).

Math (after folding the deterministic permutation skeleton + adj into the
per-channel input weights):
    W[j, c, h] = adj[j, c] * w_in[c, k(j,c), h]   (0 at j == c)
    z[n, c, :] = data[n, :] @ W[:, c, :] + b_in[c]
    out[n, c]  = sum_h tanh(z)[n, c, h] * (neurons[h, c] * w_out[c, h, 0]) + b_out[c]
    returns (adj, out)

Sharding: channel-parallel — each of the 8 cores computes 32 of the 256
channels for all 4096 samples. data.T is replicated to every core (2MB in
fp16, streamed in 512-col pieces), per-channel weights are sliced per core.

Per-core device kernel: for each 1024-wide block of samples,
  z (128p x 1024) = two fp16 matmuls (K=256 split in 2) into fp32 PSUM
  h = tanh(z + b_in) on ScalarE (PSUM -> SBUF, fp16)
  out (32p x 1024) += G^T @ h   (gate+w_out folded into a (128,32) stationary)
then + b_out on VectorE and DMA out.
"""

import sys

if "/opt/trn_rl_repo" not in sys.path:
    sys.path.insert(0, "/opt/trn_rl_repo")

import numpy as np

N, V, NH = 4096, 256, 64
NCORES = 8
CPC = V // NCORES            # 32 channels per core
M_PER_CORE = CPC * NH        # 2048 (c_local, h) columns
MT = M_PER_CORE // 128       # 16 m-tiles (2 channels each)
NBLK = 1024                  # sample columns per block (2 PSUM banks)
NB = N // NBLK               # 4 blocks

_CACHE = {}

WARMUP_MMS = 14          # dummy matmuls to warm the PE HAM during input DMA


def _build_program():
    import concourse.bacc as bacc
    import concourse.tile as tile
    import concourse.mybir as mybir
    from concourse.tile import add_dep_helper

    f32 = mybir.dt.float32
    f16 = mybir.dt.float16
    Tanh = mybir.ActivationFunctionType.Tanh
    Ident = mybir.ActivationFunctionType.Identity

    nc = bacc.Bacc("TRN2", target_bir_lowering=False, debug=False,
                   num_devices=NCORES)

    x2_d = nc.dram_tensor("x2", [128, 2 * N], f16, kind="ExternalInput").ap()
    w_d = nc.dram_tensor("w", [V, M_PER_CORE], f16, kind="ExternalInput").ap()
    g_d = nc.dram_tensor("g", [128, MT * CPC], f16, kind="ExternalInput").ap()
    b_d = nc.dram_tensor("b", [128, MT], f32, kind="ExternalInput").ap()
    bo_d = nc.dram_tensor("bo", [CPC, 1], f32, kind="ExternalInput").ap()
    out_d = nc.dram_tensor("out", [CPC, N], f32, kind="ExternalOutput").ap()

    with tile.TileContext(nc) as tc:
        with tc.tile_pool(name="const", bufs=1) as constp, \
             tc.tile_pool(name="xin", bufs=4) as xp, \
             tc.tile_pool(name="warm", bufs=1) as warmp, \
             tc.tile_pool(name="zpsum", bufs=3, space="PSUM") as zp, \
             tc.tile_pool(name="redpsum", bufs=1, space="PSUM") as rp, \
             tc.tile_pool(name="hbuf", bufs=9) as hp, \
             tc.tile_pool(name="obuf", bufs=2) as op:

            # ── PE warm-up + early tanh table load. Dummy matmuls fill the
            # initial DMA wait and flip HAM to 2.4 GHz; the tiny activation
            # forces ACT_TABLE_LOAD to run now instead of at first real tanh.
            wt_w = warmp.tile([128, 512], f16, tag="warmdat")
            nc.vector.memset(wt_w[:], 0.0)
            if WARMUP_MMS:
                wt_ps = zp.tile([128, NBLK], f32, tag="z", name="warm_ps")
                for _ in range(WARMUP_MMS):
                    nc.tensor.matmul(wt_ps[:, 0:512], lhsT=wt_w[:, 0:128],
                                     rhs=wt_w[:], start=True, stop=True)

            # ── startup DMAs. x is host-packed so one 256KB DMA delivers
            # both K-halves of a 512-sample piece. First z group needs
            # x(0,0) + w0c0 + w1c0: spread across the gpsimd/sync/scalar
            # rings (each ring's transfers serialize at ~110GB/s).
            def xtile(blk, half):
                t = xp.tile([128, NBLK], f16, tag=f"xh{half}",
                            name=f"x_{blk}_{half}")
                return t

            def xsrc(blk, half):
                cb = blk * 2 + half
                return x2_d[:, cb * NBLK:(cb + 1) * NBLK]

            # block-0 x pieces split by K-half and interleaved with the w
            # chunks across both HWDGE rings, in exact first-use order
            # (subtile deps let the first matmul start on the kt0 piece)
            x_tiles = []
            t0 = xtile(0, 0)
            t1 = xtile(0, 1)
            x_tiles.append((0, t0))
            x_tiles.append((1, t1))
            w_sb = [constp.tile([128, M_PER_CORE], f16, tag=f"w{kt}",
                                name=f"w_sb{kt}")
                    for kt in range(2)]
            WCH = 1024
            nc.sync.dma_start(t0[:, 0:512], x2_d[:, 0:512])
            nc.scalar.dma_start(t0[:, 512:NBLK], x2_d[:, 512:NBLK])
            nc.sync.dma_start(w_sb[0][:, 0:WCH], w_d[0:128, 0:WCH])
            nc.scalar.dma_start(w_sb[1][:, 0:WCH], w_d[128:256, 0:WCH])
            nc.sync.dma_start(t1[:, 0:512], x2_d[:, NBLK:NBLK + 512])
            nc.scalar.dma_start(t1[:, 512:NBLK], x2_d[:, NBLK + 512:2 * NBLK])
            for ch in range(1, M_PER_CORE // WCH):
                csl = slice(ch * WCH, (ch + 1) * WCH)
                nc.sync.dma_start(w_sb[0][:, csl], w_d[0:128, csl])
                nc.scalar.dma_start(w_sb[1][:, csl], w_d[128:256, csl])
            b_sb = constp.tile([128, MT], f32, tag="b")
            nc.gpsimd.dma_start(b_sb[:], b_d[:])
            bo_sb = constp.tile([CPC, 1], f32, tag="bo")
            nc.gpsimd.dma_start(bo_sb[:], bo_d[:])
            g_sb = constp.tile([128, MT * CPC], f16, tag="g")
            nc.gpsimd.dma_start(g_sb[:], g_d[:])

            # reduction matmuls run RED_LAG mt-steps behind their tanh so
            # the PE never waits on ScalarE
            RED_LAG = 3
            pending_red = []

            def emit_red(pr):
                h, gsl, first, last, red_t, pblk, pnsl_d = pr
                for half in range(NBLK // 512):
                    nsl = slice(half * 512, (half + 1) * 512)
                    nc.tensor.matmul(red_t[:, nsl],
                                     lhsT=g_sb[:, gsl],
                                     rhs=h[:, nsl],
                                     start=first, stop=last)
                if last:
                    o = op.tile([CPC, NBLK], f32, tag="o")
                    nc.vector.tensor_scalar_add(o[:], red_t[:], bo_sb[:, 0:1])
                    nc.sync.dma_start(out_d[:, pnsl_d], o[:])

            for blk in range(NB):
                nsl_d = slice(blk * NBLK, (blk + 1) * NBLK)
                xr = x_tiles
                red = rp.tile([CPC, NBLK], f32)
                for mt in range(MT):
                    msl = slice(mt * 128, (mt + 1) * 128)
                    z = zp.tile([128, NBLK], f32, tag="z")
                    xmap = {hf: t for hf, t in xr}
                    for half in range(NBLK // 512):
                        nsl = slice(half * 512, (half + 1) * 512)
                        nc.tensor.matmul(z[:, nsl],
                                         lhsT=w_sb[0][:, msl],
                                         rhs=xmap[half][:, 0:512],
                                         start=True, stop=False)
                        nc.tensor.matmul(z[:, nsl],
                                         lhsT=w_sb[1][:, msl],
                                         rhs=xmap[half][:, 512:NBLK],
                                         start=False, stop=True)
                    # drain reds in batches of 4 so the PE pays the w<->g
                    # stationary-switch cost only twice per 4 mt-steps; in the
                    # last block drain everything ready at mt14 so only mt15's
                    # reduction trails the final tanh
                    if blk == NB - 1 and mt == MT - 1:
                        while pending_red:
                            emit_red(pending_red.pop(0))
                    elif len(pending_red) >= RED_LAG + 3:
                        for _ in range(4):
                            emit_red(pending_red.pop(0))
                    h = hp.tile([128, NBLK], f16)
                    act_inst = nc.scalar.activation(h[:], z[:], Tanh,
                                                    bias=b_sb[:, mt:mt + 1])
                    if mt == 1:
                        prefetch_gate = act_inst
                    pending_red.append((h, slice(mt * CPC, (mt + 1) * CPC),
                                        mt == 0, mt == MT - 1, red, blk,
                                        nsl_d))

                # prefetch next block's x on the gpsimd queue
                if blk + 1 < NB:
                    x_tiles = []
                    for half in range(2):
                        t = xtile(blk + 1, half)
                        dma = nc.gpsimd.dma_start(t[:], xsrc(blk + 1, half))
                        # hold the prefetch back so it can't steal HBM
                        # bandwidth from startup-critical loads
                        add_dep_helper(dma.ins, prefetch_gate.ins,
                                       reason="x prefetch after block start")
                        x_tiles.append((half, t))

            for pr in pending_red:
                emit_red(pr)
            pending_red = []

    nc.compile()
    return nc


def _get_program():
    if "nc" not in _CACHE:
        _CACHE["nc"] = _build_program()
    return _CACHE["nc"]


def _make_in_maps(data, adj, w_in, b_in, w_out, neurons, b_out):
    xT = data.T.astype(np.float16)  # (V, N)
    # pack per-(block,half) chunks as [kt0 512 cols | kt1 512 cols]
    x2 = np.empty((128, 2 * N), dtype=np.float16)
    for blk in range(NB):
        for half in range(2):
            cb = blk * 2 + half
            seg = slice(blk * NBLK + half * 512, blk * NBLK + (half + 1) * 512)
            x2[:, cb * NBLK:cb * NBLK + 512] = xT[0:128, seg]
            x2[:, cb * NBLK + 512:(cb + 1) * NBLK] = xT[128:256, seg]
    x2 = np.ascontiguousarray(x2)

    # Fold adj + permutation skeleton into dense per-channel weights.
    W = np.zeros((V, V, NH), dtype=np.float32)  # (j, c, h)
    for c in range(V):
        ac = adj[:, c]
        W[:c, c, :] = w_in[c, :c, :] * ac[:c, None]
        W[c + 1:, c, :] = w_in[c, c:, :] * ac[c + 1:, None]

    gate = neurons.T * w_out[:, :, 0]  # (V, NH)

    in_maps = []
    for k in range(NCORES):
        cs = slice(CPC * k, CPC * (k + 1))
        Wk = np.ascontiguousarray(
            W[:, cs, :].reshape(V, M_PER_CORE).astype(np.float16))
        gk = gate[cs]     # (32, 64)
        bik = b_in[cs]    # (32, 64)
        Gk = np.zeros((128, MT * CPC), dtype=np.float32)
        bk = np.zeros((128, MT), dtype=np.float32)
        for mt in range(MT):
            for i in range(2):
                cl = 2 * mt + i
                Gk[i * NH:(i + 1) * NH, mt * CPC + cl] = gk[cl]
                bk[i * NH:(i + 1) * NH, mt] = bik[cl]
        bok = np.ascontiguousarray(b_out[cs].reshape(CPC, 1).astype(np.float32))
        in_maps.append({"x2": x2, "w": Wk, "g": Gk.astype(np.float16),
                        "b": bk, "bo": bok})
    return in_maps


def _run(in_maps, trace=False, tmpdir=None):
    from concourse.bass_utils import run_bass_kernel_spmd
    nc = _get_program()
    return run_bass_kernel_spmd(nc, in_maps, core_ids=list(range(NCORES)),
                                trace=trace, tmpdir=tmpdir)


def kernel(data, adj, neurons, w_in, b_in, w_out, b_out, perm):
    data = np.asarray(data, dtype=np.float32)
    adj = np.asarray(adj, dtype=np.float32)
    neurons = np.asarray(neurons, dtype=np.float32)
    w_in = np.asarray(w_in, dtype=np.float32)
    b_in = np.asarray(b_in, dtype=np.float32)
    w_out = np.asarray(w_out, dtype=np.float32)
    b_out = np.asarray(b_out, dtype=np.float32)
    assert data.shape == (N, V)

    in_maps = _make_in_maps(data, adj, w_in, b_in, w_out, neurons, b_out)
    res = _run(in_maps)

    out = np.empty((N, V), dtype=np.float32)
    for k in range(NCORES):
        out[:, CPC * k:CPC * (k + 1)] = res.results[k]["out"].T
    return adj, out


# revision 46
# speedup vs baseline: 1.0126x; 1.0043x over previous
"""Trainium2 Bass kernel for nn_CustomizedRelaModule (gnn_message_passing).

Math (after folding the deterministic permutation skeleton + adj into the
per-channel input weights):
    W[j, c, h] = adj[j, c] * w_in[c, k(j,c), h]   (0 at j == c)
    z[n, c, :] = data[n, :] @ W[:, c, :] + b_in[c]
    out[n, c]  = sum_h tanh(z)[n, c, h] * (neurons[h, c] * w_out[c, h, 0]) + b_out[c]
    returns (adj, out)

Sharding: channel-parallel — each of the 8 cores computes 32 of the 256
channels for all 4096 samples. data.T is replicated to every core (2MB in
fp16, streamed in 512-col pieces), per-channel weights are sliced per core.

Per-core device kernel: for each 1024-wide block of samples,
  z (128p x 1024) = two fp16 matmuls (K=256 split in 2) into fp32 PSUM
  h = tanh(z + b_in) on ScalarE (PSUM -> SBUF, fp16)
  out (32p x 1024) += G^T @ h   (gate+w_out folded into a (128,32) stationary)
then + b_out on VectorE and DMA out.
"""

import sys

if "/opt/trn_rl_repo" not in sys.path:
    sys.path.insert(0, "/opt/trn_rl_repo")

import numpy as np

N, V, NH = 4096, 256, 64
NCORES = 8
CPC = V // NCORES            # 32 channels per core
M_PER_CORE = CPC * NH        # 2048 (c_local, h) columns
MT = M_PER_CORE // 128       # 16 m-tiles (2 channels each)
NBLK = 1024                  # sample columns per block (2 PSUM banks)
NB = N // NBLK               # 4 blocks

_CACHE = {}

WARMUP_MMS = 14          # dummy matmuls to warm the PE HAM during input DMA


def _build_program():
    import concourse.bacc as bacc
    import concourse.tile as tile
    import concourse.mybir as mybir
    from concourse.tile import add_dep_helper

    f32 = mybir.dt.float32
    f16 = mybir.dt.float16
    Tanh = mybir.ActivationFunctionType.Tanh
    Ident = mybir.ActivationFunctionType.Identity

    nc = bacc.Bacc("TRN2", target_bir_lowering=False, debug=False,
                   num_devices=NCORES)

    x2_d = nc.dram_tensor("x2", [128, 2 * N], f16, kind="ExternalInput").ap()
    w_d = nc.dram_tensor("w", [V, M_PER_CORE], f16, kind="ExternalInput").ap()
    g_d = nc.dram_tensor("g", [128, MT * CPC], f16, kind="ExternalInput").ap()
    b_d = nc.dram_tensor("b", [128, MT], f32, kind="ExternalInput").ap()
    bo_d = nc.dram_tensor("bo", [CPC, 1], f32, kind="ExternalInput").ap()
    out_d = nc.dram_tensor("out", [CPC, N], f32, kind="ExternalOutput").ap()

    with tile.TileContext(nc) as tc:
        with tc.tile_pool(name="const", bufs=1) as constp, \
             tc.tile_pool(name="xin", bufs=4) as xp, \
             tc.tile_pool(name="warm", bufs=1) as warmp, \
             tc.tile_pool(name="zpsum", bufs=3, space="PSUM") as zp, \
             tc.tile_pool(name="redpsum", bufs=1, space="PSUM") as rp, \
             tc.tile_pool(name="hbuf", bufs=9) as hp, \
             tc.tile_pool(name="obuf", bufs=2) as op:

            # ── PE warm-up + early tanh table load. Dummy matmuls fill the
            # initial DMA wait and flip HAM to 2.4 GHz; the tiny activation
            # forces ACT_TABLE_LOAD to run now instead of at first real tanh.
            wt_w = warmp.tile([128, 512], f16, tag="warmdat")
            nc.vector.memset(wt_w[:], 0.0)
            if WARMUP_MMS:
                wt_ps = zp.tile([128, NBLK], f32, tag="z", name="warm_ps")
                for _ in range(WARMUP_MMS):
                    nc.tensor.matmul(wt_ps[:, 0:512], lhsT=wt_w[:, 0:128],
                                     rhs=wt_w[:], start=True, stop=True)

            # ── startup DMAs. x is host-packed so one 256KB DMA delivers
            # both K-halves of a 512-sample piece. First z group needs
            # x(0,0) + w0c0 + w1c0: spread across the gpsimd/sync/scalar
            # rings (each ring's transfers serialize at ~110GB/s).
            def xtile(blk, half):
                t = xp.tile([128, NBLK], f16, tag=f"xh{half}",
                            name=f"x_{blk}_{half}")
                return t

            def xsrc(blk, half):
                cb = blk * 2 + half
                return x2_d[:, cb * NBLK:(cb + 1) * NBLK]

            # block-0 x pieces split by K-half and interleaved with the w
            # chunks across both HWDGE rings, in exact first-use order
            # (subtile deps let the first matmul start on the kt0 piece)
            x_tiles = []
            t0 = xtile(0, 0)
            t1 = xtile(0, 1)
            x_tiles.append((0, t0))
            x_tiles.append((1, t1))
            w_sb = [constp.tile([128, M_PER_CORE], f16, tag=f"w{kt}",
                                name=f"w_sb{kt}")
                    for kt in range(2)]
            WCH = 1024
            nc.sync.dma_start(t0[:, 0:512], x2_d[:, 0:512])
            nc.scalar.dma_start(t0[:, 512:NBLK], x2_d[:, 512:NBLK])
            nc.sync.dma_start(w_sb[0][:, 0:WCH], w_d[0:128, 0:WCH])
            nc.scalar.dma_start(w_sb[1][:, 0:WCH], w_d[128:256, 0:WCH])
            nc.sync.dma_start(t1[:, 0:512], x2_d[:, NBLK:NBLK + 512])
            nc.scalar.dma_start(t1[:, 512:NBLK], x2_d[:, NBLK + 512:2 * NBLK])
            for ch in range(1, M_PER_CORE // WCH):
                csl = slice(ch * WCH, (ch + 1) * WCH)
                nc.sync.dma_start(w_sb[0][:, csl], w_d[0:128, csl])
                nc.scalar.dma_start(w_sb[1][:, csl], w_d[128:256, csl])
            b_sb = constp.tile([128, MT], f32, tag="b")
            nc.gpsimd.dma_start(b_sb[:], b_d[:])
            bo_sb = constp.tile([CPC, 1], f32, tag="bo")
            nc.gpsimd.dma_start(bo_sb[:], bo_d[:])
            g_sb = constp.tile([128, MT * CPC], f16, tag="g")
            nc.gpsimd.dma_start(g_sb[:], g_d[:])

            # reduction matmuls run RED_LAG mt-steps behind their tanh so
            # the PE never waits on ScalarE
            RED_LAG = 3
            pending_red = []

            def emit_red(pr):
                h, gsl, first, last, red_t, pblk, pnsl_d = pr
                for half in range(NBLK // 512):
                    nsl = slice(half * 512, (half + 1) * 512)
                    nc.tensor.matmul(red_t[:, nsl],
                                     lhsT=g_sb[:, gsl],
                                     rhs=h[:, nsl],
                                     start=first, stop=last)
                if last:
                    o = op.tile([CPC, NBLK], f32, tag="o")
                    nc.vector.tensor_scalar_add(o[:], red_t[:], bo_sb[:, 0:1])
                    # split the output store across both HWDGE rings so the
                    # tail transfer runs at 2x ring bandwidth
                    nc.sync.dma_start(out_d[:, pnsl_d.start:pnsl_d.start + 512],
                                      o[:, 0:512])
                    nc.scalar.dma_start(out_d[:, pnsl_d.start + 512:pnsl_d.stop],
                                        o[:, 512:NBLK])

            for blk in range(NB):
                nsl_d = slice(blk * NBLK, (blk + 1) * NBLK)
                xr = x_tiles
                red = rp.tile([CPC, NBLK], f32)
                for mt in range(MT):
                    msl = slice(mt * 128, (mt + 1) * 128)
                    z = zp.tile([128, NBLK], f32, tag="z")
                    xmap = {hf: t for hf, t in xr}
                    for half in range(NBLK // 512):
                        nsl = slice(half * 512, (half + 1) * 512)
                        nc.tensor.matmul(z[:, nsl],
                                         lhsT=w_sb[0][:, msl],
                                         rhs=xmap[half][:, 0:512],
                                         start=True, stop=False)
                        nc.tensor.matmul(z[:, nsl],
                                         lhsT=w_sb[1][:, msl],
                                         rhs=xmap[half][:, 512:NBLK],
                                         start=False, stop=True)
                    # drain reds in batches of 4 so the PE pays the w<->g
                    # stationary-switch cost only twice per 4 mt-steps; in the
                    # last block drain everything ready at mt14 so only mt15's
                    # reduction trails the final tanh
                    if blk == NB - 1 and mt == MT - 1:
                        while pending_red:
                            emit_red(pending_red.pop(0))
                    elif len(pending_red) >= RED_LAG + 3:
                        for _ in range(4):
                            emit_red(pending_red.pop(0))
                    h = hp.tile([128, NBLK], f16)
                    act_inst = nc.scalar.activation(h[:], z[:], Tanh,
                                                    bias=b_sb[:, mt:mt + 1])
                    if mt == 1:
                        prefetch_gate = act_inst
                    pending_red.append((h, slice(mt * CPC, (mt + 1) * CPC),
                                        mt == 0, mt == MT - 1, red, blk,
                                        nsl_d))

                # prefetch next block's x on the gpsimd queue
                if blk + 1 < NB:
                    x_tiles = []
                    for half in range(2):
                        t = xtile(blk + 1, half)
                        dma = nc.gpsimd.dma_start(t[:], xsrc(blk + 1, half))
                        # hold the prefetch back so it can't steal HBM
                        # bandwidth from startup-critical loads
                        add_dep_helper(dma.ins, prefetch_gate.ins,
                                       reason="x prefetch after block start")
                        x_tiles.append((half, t))

            for pr in pending_red:
                emit_red(pr)
            pending_red = []

    nc.compile()
    return nc


def _get_program():
    if "nc" not in _CACHE:
        _CACHE["nc"] = _build_program()
    return _CACHE["nc"]


def _make_in_maps(data, adj, w_in, b_in, w_out, neurons, b_out):
    xT = data.T.astype(np.float16)  # (V, N)
    # pack per-(block,half) chunks as [kt0 512 cols | kt1 512 cols]
    x2 = np.empty((128, 2 * N), dtype=np.float16)
    for blk in range(NB):
        for half in range(2):
            cb = blk * 2 + half
            seg = slice(blk * NBLK + half * 512, blk * NBLK + (half + 1) * 512)
            x2[:, cb * NBLK:cb * NBLK + 512] = xT[0:128, seg]
            x2[:, cb * NBLK + 512:(cb + 1) * NBLK] = xT[128:256, seg]
    x2 = np.ascontiguousarray(x2)

    # Fold adj + permutation skeleton into dense per-channel weights.
    W = np.zeros((V, V, NH), dtype=np.float32)  # (j, c, h)
    for c in range(V):
        ac = adj[:, c]
        W[:c, c, :] = w_in[c, :c, :] * ac[:c, None]
        W[c + 1:, c, :] = w_in[c, c:, :] * ac[c + 1:, None]

    gate = neurons.T * w_out[:, :, 0]  # (V, NH)

    in_maps = []
    for k in range(NCORES):
        cs = slice(CPC * k, CPC * (k + 1))
        Wk = np.ascontiguousarray(
            W[:, cs, :].reshape(V, M_PER_CORE).astype(np.float16))
        gk = gate[cs]     # (32, 64)
        bik = b_in[cs]    # (32, 64)
        Gk = np.zeros((128, MT * CPC), dtype=np.float32)
        bk = np.zeros((128, MT), dtype=np.float32)
        for mt in range(MT):
            for i in range(2):
                cl = 2 * mt + i
                Gk[i * NH:(i + 1) * NH, mt * CPC + cl] = gk[cl]
                bk[i * NH:(i + 1) * NH, mt] = bik[cl]
        bok = np.ascontiguousarray(b_out[cs].reshape(CPC, 1).astype(np.float32))
        in_maps.append({"x2": x2, "w": Wk, "g": Gk.astype(np.float16),
                        "b": bk, "bo": bok})
    return in_maps


def _run(in_maps, trace=False, tmpdir=None):
    from concourse.bass_utils import run_bass_kernel_spmd
    nc = _get_program()
    return run_bass_kernel_spmd(nc, in_maps, core_ids=list(range(NCORES)),
                                trace=trace, tmpdir=tmpdir)


def kernel(data, adj, neurons, w_in, b_in, w_out, b_out, perm):
    data = np.asarray(data, dtype=np.float32)
    adj = np.asarray(adj, dtype=np.float32)
    neurons = np.asarray(neurons, dtype=np.float32)
    w_in = np.asarray(w_in, dtype=np.float32)
    b_in = np.asarray(b_in, dtype=np.float32)
    w_out = np.asarray(w_out, dtype=np.float32)
    b_out = np.asarray(b_out, dtype=np.float32)
    assert data.shape == (N, V)

    in_maps = _make_in_maps(data, adj, w_in, b_in, w_out, neurons, b_out)
    res = _run(in_maps)

    out = np.empty((N, V), dtype=np.float32)
    for k in range(NCORES):
        out[:, CPC * k:CPC * (k + 1)] = res.results[k]["out"].T
    return adj, out
